# revision 36
# baseline (speedup 1.0000x reference)
"""Binary complex conv (BC conv) on 8 TRN2 NeuronCores.

Reference computation:
    xb = sign(x + 1e-6)                      # (16, 256, 112, 112)
    wr = sign(weight_real + 1e-6)            # (128, 128, 3, 3)
    wi = sign(weight_imag + 1e-6)
    kernel = [[wr, -wi], [wi, wr]]           # (256, 256, 3, 3)
    out = conv2d(xb, kernel, pad=1) + bias   # (16, 256, 112, 112)

Strategy: pure data-parallel over batch (2 images per core); everything
else on-device, numerically exact (all matmul operands are +-1/0/+-2 ->
exact in fp8e4/bf16; PSUM accumulates fp32).

Two tricks on top of the direct conv:
 * Karatsuba for the complex structure: A = xr*wr, B = xi*wi,
   C = (xr+xi)*(wr+wi); out_real = A-B, out_imag = C-A-B.
   3 convs of 128 input channels instead of 4.
 * fp8 DoubleRow: each binarized frame is stored with row stride 114;
   conv taps in raster order have flat offsets [-115,-114,-113,-1,0,1,
   113,114,115], so consecutive taps pair into DoubleRow matmuls
   (contraction 256) with pair strides 1/112/1/1 + one normal matmul.

Each 4-output-row tile accumulates into a [128, 456] PSUM bank
(garbage pad lanes skipped on eviction).
"""

import numpy as np

import concourse.bass as bass
import concourse.tile as tile
from concourse import mybir
from concourse.bass_utils import run_bass_kernel_spmd

N_CORES = 8
B = 16
CPB = 128          # channels per block (partition dim)
H = W = 112
RS = 114           # frame row stride
FROWS = 116        # 114 padded rows + 2 junk margin rows
IMGS = 2
TROWS = 4          # output rows per matmul tile
NT = TROWS * RS    # matmul free dim (456)
NTILES = H // TROWS
BAND = 28          # input rows binarized per activation op
EPS = 1e-6

F32 = mybir.dt.float32
FP8 = mybir.dt.float8e4
AF = mybir.ActivationFunctionType
DRM = mybir.MatmulPerfMode.DoubleRow
ALU = mybir.AluOpType

# tap flat offsets in raster order; pairs (0,1) (2,3) (4,5) (6,7), single 8
TAP_OFF = [dy * RS + dx for dy in (-1, 0, 1) for dx in (-1, 0, 1)]


def _split_multiwait(nc):
    """Walrus in this container rejects >1 semaphore wait per instruction
    ("Too many sync wait commands"); hoist extra waits onto preceding nops
    on the same engine."""
    import bass_rust

    for f in nc.m.functions:
        for bb in f.blocks:
            new_insts = []
            for inst in bb.instructions:
                si = inst.sync_info
                waits = list(si.on_wait) if si is not None and si.on_wait else []
                if len(waits) > 1:
                    for w in waits[:-1]:
                        nop = mybir.InstNoOp(
                            name=nc.get_next_instruction_name(),
                            engine=inst.engine,
                            ins=[],
                            outs=[],
                        )
                        nop.sync_info = bass_rust.SyncInfo(on_wait=[w], on_update=[])
                        new_insts.append(nop)
                    si.on_wait = [waits[-1]]
                    inst.sync_info = si
                new_insts.append(inst)
            bb.instructions = new_insts


def build_nc():
    nc = bass.Bass()

    x_ext = nc.declare_dram_parameter("x", [IMGS, 2 * CPB, H, W], F32, isOutput=False)
    wr_ext = nc.declare_dram_parameter("wrT", [CPB, 9 * CPB], F32, isOutput=False)
    wi_ext = nc.declare_dram_parameter("wiT", [CPB, 9 * CPB], F32, isOutput=False)
    bias_ext = nc.declare_dram_parameter("bias2", [CPB, 2], F32, isOutput=False)
    out_ext = nc.declare_dram_parameter("out", [IMGS, 2 * CPB, H, W], F32, isOutput=True)

    x_flat = x_ext.rearrange("b c h w -> (b c) h w")       # [512, 112, 112]
    out_flat = out_ext.rearrange("b c h w -> (b c) h w")

    with tile.TileContext(nc) as tc:
        with (
            tc.tile_pool(name="wstage", bufs=2) as wstage_pool,
            tc.tile_pool(name="wbin", bufs=1) as wbin_pool,
            tc.tile_pool(name="biasp", bufs=1) as bias_pool,
            tc.tile_pool(name="xq", bufs=1) as xq_pool,
            tc.tile_pool(name="stage", bufs=4) as stage_pool,
            tc.tile_pool(name="tmp", bufs=6) as tmp_pool,
            tc.tile_pool(name="outsb", bufs=8) as out_pool,
            tc.tile_pool(name="psum", bufs=8, space="PSUM") as psum_pool,
        ):
            # per-partition scalar constant for activation bias
            eps_pos = bias_pool.tile([CPB, 1], F32, tag="epsp")
            nc.gpsimd.memset(eps_pos[:], EPS)

            # HAM warmup: dummy matmuls on junk data with no dependencies so
            # the PE clock-gate reaches 8/8 before the first real matmul
            junk = bias_pool.tile([CPB, 512], FP8, tag="junk")
            nc.gpsimd.memset(junk[:, 0:1], 1.0)
            jps = psum_pool.tile([CPB, 512], F32, tag="ps", name="jps")
            for _ in range(12):
                nc.tensor.matmul(jps[:], junk[:, :CPB], junk[:], start=True,
                                 stop=True)
            for _ in range(70):
                nc.tensor.matmul(jps[:, :256], junk[:, :CPB], junk[:, :256],
                                 start=True, stop=True)
            jout = bias_pool.tile([CPB, 1], F32, tag="jout")
            nc.vector.tensor_copy(jout[:], jps[:, 0:1])

            # ---- weights ----
            wr_f32 = wstage_pool.tile([CPB, 9 * CPB], F32, tag="wstage")
            nc.scalar.dma_start(wr_f32[:, :576], wr_ext[:, :576])
            nc.scalar.dma_start(wr_f32[:, 576:], wr_ext[:, 576:])
            wi_f32 = wstage_pool.tile([CPB, 9 * CPB], F32, tag="wstage")
            nc.scalar.dma_start(wi_f32[:, :576], wi_ext[:, :576])
            nc.scalar.dma_start(wi_f32[:, 576:], wi_ext[:, 576:])

            # binarized fp8 weights [ci, tap, co]; wq_s = wq_r + wq_i
            wq_r = wbin_pool.tile([CPB, 9, CPB], FP8, tag="wqr")
            wq_i = wbin_pool.tile([CPB, 9, CPB], FP8, tag="wqi")
            wq_s = wbin_pool.tile([CPB, 9, CPB], FP8, tag="wqs")
            wr_v = wr_f32[:].rearrange("p (t c) -> p t c", c=CPB)
            wi_v = wi_f32[:].rearrange("p (t c) -> p t c", c=CPB)
            nc.scalar.activation(wq_r[:], wr_v, AF.Sign, bias=eps_pos[:], scale=1.0)

            bias_sb = bias_pool.tile([CPB, 2], F32)
            nc.sync.dma_start(bias_sb[:], bias_ext[:])
            bias_ir = bias_pool.tile([CPB, 1], F32, tag="biasir")
            nc.vector.tensor_add(bias_ir[:], bias_sb[:, 1:2], bias_sb[:, 0:1])

            # ---- persistent binarized fp8 frames ----
            # frame: [128, FROWS, RS]; frame row = padded row + 1 (1 junk
            # margin row on top); cols 0 / 113 are the zero pad columns,
            # cols 114-115 slack (only ever read into discarded pad lanes)
            def frame(nm):
                return xq_pool.tile([CPB, FROWS, RS], FP8, tag=nm, name=nm)

            xqr = [frame(f"xqr{i}") for i in range(IMGS)]
            xqi = [frame(f"xqi{i}") for i in range(IMGS)]
            xqs = [frame(f"xqs{i}") for i in range(IMGS)]
            for i in range(IMGS):
                eng = nc.vector if i == 0 else nc.gpsimd
                for t in (xqr[i], xqi[i], xqs[i]):
                    eng.memset(t[:, 1:2, :], 0.0)          # padded row 0
                    eng.memset(t[:, 114:115, :], 0.0)      # padded row 113
                    eng.memset(t[:, 1:115, 0:1], 0.0)      # padded col 0
                    eng.memset(t[:, 1:115, 113:114], 0.0)  # padded col 113

            flat = {}
            for i in range(IMGS):
                flat[("r", i)] = xqr[i][:].rearrange("p r c -> p (r c)")
                flat[("i", i)] = xqi[i][:].rearrange("p r c -> p (r c)")
                flat[("s", i)] = xqs[i][:].rearrange("p r c -> p (r c)")

            # ---- binarize input + build the sum frame, band by band ----
            def binarize_rows(img, r0, nr):
                rows = slice(r0 + 2, r0 + 2 + nr)
                for cib, dst in ((0, xqr), (1, xqi)):
                    ch0 = img * 2 * CPB + cib * CPB
                    st = stage_pool.tile([CPB, BAND, W], F32, tag="stage")
                    nc.sync.dma_start(
                        st[:, :nr, :],
                        x_flat[ch0:ch0 + CPB, r0:r0 + nr, :],
                    )
                    nc.scalar.activation(
                        dst[img][:, rows, 1:113], st[:, :nr, :],
                        AF.Sign, bias=eps_pos[:], scale=1.0,
                    )
                nc.vector.tensor_tensor(
                    xqs[img][:, rows, 1:113],
                    xqr[img][:, rows, 1:113],
                    xqi[img][:, rows, 1:113],
                    op=ALU.add,
                )

            def binarize_band(img, b, strips=1):
                r0 = b * BAND
                step = BAND // strips
                for s in range(strips):
                    binarize_rows(img, r0 + s * step, step)

            def conv_tiles(img, tiles):
                for t in tiles:
                    base = (4 * t + 2) * RS
                    pk = {}
                    def conv(kind):
                        w3 = {"r": wq_r, "i": wq_i, "s": wq_s}[kind]
                        xf = flat[(kind, img)]
                        ps = psum_pool.tile([CPB, NT], F32, tag="ps",
                                            name=f"ps_{kind}{img}_{t}")
                        pk[kind] = ps
                        part = [list(xf.ap)[0][0], CPB]
                        for p in range(4):
                            o0, o1 = TAP_OFF[2 * p], TAP_OFF[2 * p + 1]
                            rhs = bass.AP(
                                xf.tensor, xf.offset + o0 + base,
                                [part, [o1 - o0, 2], [1, NT]],
                            )
                            nc.tensor.matmul(
                                ps[:], w3[:, 2 * p:2 * p + 2, :], rhs,
                                start=(p == 0), stop=False, perf_mode=DRM,
                            )
                        nc.tensor.matmul(
                            ps[:], w3[:, 8, :],
                            xf[:, base + TAP_OFF[8]:base + TAP_OFF[8] + NT],
                            start=False, stop=True,
                        )
                        return ps

                    # out_real = A - B + bias_r ; out_imag = C - A - B + bias_i
                    # ScalarE (fast PSUM port) evacuates each bank compactly
                    # right after its conv, exactly one reader per bank:
                    #   An2 = A + bias_r ; Bn0 = -B
                    # then out_real = An2 + Bn0 (SBUF-only, GpSimd)
                    #      out_imag = ((C - An2) + (bias_i+bias_r)) + Bn0
                    A = conv("r")
                    An2 = tmp_pool.tile([CPB, TROWS, W], F32, tag="An")
                    Av = A[:].rearrange("p (r c) -> p r c", c=RS)
                    nc.scalar.activation(An2[:], Av[:, :, 1:113], AF.Identity,
                                         bias=bias_sb[:, 0:1], scale=1.0)
                    Bp = conv("i")
                    Bn0 = tmp_pool.tile([CPB, TROWS, W], F32, tag="Bn")
                    Bv = Bp[:].rearrange("p (r c) -> p r c", c=RS)
                    nc.scalar.activation(Bn0[:], Bv[:, :, 1:113], AF.Identity,
                                         bias=0.0, scale=-1.0)
                    C = conv("s")
                    Cv = C[:].rearrange("p (r c) -> p r c", c=RS)

                    osb = out_pool.tile([CPB, 2, TROWS, W], F32, tag="osb")
                    nc.gpsimd.tensor_tensor(osb[:, 0], An2[:], Bn0[:], op=ALU.add)
                    t5 = tmp_pool.tile([CPB, TROWS, W], F32, tag="t5")
                    nc.vector.tensor_sub(t5[:], Cv[:, :, 1:113], An2[:])
                    nc.vector.scalar_tensor_tensor(
                        osb[:, 1], t5[:], bias_ir[:], Bn0[:],
                        op0=ALU.add, op1=ALU.add,
                    )

                    # one DMA for both channel halves: dst walks [ch-within-
                    # block, block, row, col] to match the tile's layout
                    dst = bass.AP(
                        out_flat.tensor,
                        img * 2 * CPB * H * W + 4 * t * W,
                        [[H * W, CPB], [CPB * H * W, 2], [W, TROWS], [1, W]],
                    )
                    nc.sync.dma_start(dst, osb[:])

            # tile t needs input rows <= 4t+4; band b supplies rows < 28(b+1).
            # Binarize lands in 14-row strips, interleaved BETWEEN tiles so
            # the long Sign ops never head-of-line-block the short PSUM
            # evacuations in the static ScalarE queue; strips stay three
            # tile-groups ahead of their consumers.
            ranges = [range(0, 6), range(6, 13), range(13, 20), range(20, 28)]
            groups = [(i, b) for i in range(IMGS) for b in range(H // BAND)]
            binarize_rows(0, 0, 14)
            nc.scalar.activation(wq_i[:], wi_v, AF.Sign, bias=eps_pos[:], scale=1.0)
            nc.vector.tensor_tensor(wq_s[:], wq_r[:], wq_i[:], op=ALU.add)
            binarize_rows(0, 14, 14)
            for b in (1, 2):
                binarize_band(0, b, strips=2)
            for gi, (img, b) in enumerate(groups):
                tiles = list(ranges[b])
                ahead = groups[gi + 3] if gi + 3 < len(groups) else None
                conv_tiles(img, tiles[:2])
                if ahead:
                    binarize_rows(ahead[0], ahead[1] * BAND, 14)
                conv_tiles(img, tiles[2:4])
                if ahead:
                    binarize_rows(ahead[0], ahead[1] * BAND + 14, 14)
                conv_tiles(img, tiles[4:])

    _split_multiwait(nc)
    return nc


def _prep(x, weight_real, weight_imag, bias):
    x = np.ascontiguousarray(np.asarray(x, dtype=np.float32))
    wr = np.asarray(weight_real, dtype=np.float32)
    wi = np.asarray(weight_imag, dtype=np.float32)
    bias = np.asarray(bias, dtype=np.float32)
    wrT = np.ascontiguousarray(wr.transpose(1, 2, 3, 0).reshape(CPB, 9 * CPB))
    wiT = np.ascontiguousarray(wi.transpose(1, 2, 3, 0).reshape(CPB, 9 * CPB))
    bias2 = np.ascontiguousarray(bias.reshape(2, CPB).T)
    return [
        {"x": x[IMGS * c:IMGS * (c + 1)], "wrT": wrT, "wiT": wiT, "bias2": bias2}
        for c in range(N_CORES)
    ]


def kernel(x, weight_real, weight_imag, bias):
    in_maps = _prep(x, weight_real, weight_imag, bias)
    nc = build_nc()
    res = run_bass_kernel_spmd(nc, in_maps, core_ids=list(range(N_CORES)))
    return np.concatenate([res.results[i]["out"] for i in range(N_CORES)], axis=0)


def run_traced(x, weight_real, weight_imag, bias, **trace_kwargs):
    """test.py entry: same as kernel() but with neuron-profile tracing."""
    in_maps = _prep(x, weight_real, weight_imag, bias)
    nc = build_nc()
    res = run_bass_kernel_spmd(
        nc, in_maps, core_ids=list(range(N_CORES)), trace=True, **trace_kwargs
    )
    out = np.concatenate([res.results[i]["out"] for i in range(N_CORES)], axis=0)
    return out, res


# revision 37
# speedup vs baseline: 1.0218x; 1.0218x over previous
"""Binary complex conv (BC conv) on 8 TRN2 NeuronCores.

Reference computation:
    xb = sign(x + 1e-6)                      # (16, 256, 112, 112)
    wr = sign(weight_real + 1e-6)            # (128, 128, 3, 3)
    wi = sign(weight_imag + 1e-6)
    kernel = [[wr, -wi], [wi, wr]]           # (256, 256, 3, 3)
    out = conv2d(xb, kernel, pad=1) + bias   # (16, 256, 112, 112)

Strategy: pure data-parallel over batch (2 images per core); everything
else on-device, numerically exact (all matmul operands are +-1/0/+-2 ->
exact in fp8e4/bf16; PSUM accumulates fp32).

Two tricks on top of the direct conv:
 * Karatsuba for the complex structure: A = xr*wr, B = xi*wi,
   C = (xr+xi)*(wr+wi); out_real = A-B, out_imag = C-A-B.
   3 convs of 128 input channels instead of 4.
 * fp8 DoubleRow: each binarized frame is stored with row stride 114;
   conv taps in raster order have flat offsets [-115,-114,-113,-1,0,1,
   113,114,115], so consecutive taps pair into DoubleRow matmuls
   (contraction 256) with pair strides 1/112/1/1 + one normal matmul.

Each 4-output-row tile accumulates into a [128, 456] PSUM bank
(garbage pad lanes skipped on eviction).
"""

import numpy as np

import concourse.bass as bass
import concourse.tile as tile
from concourse import mybir
from concourse.bass_utils import run_bass_kernel_spmd

N_CORES = 8
B = 16
CPB = 128          # channels per block (partition dim)
H = W = 112
RS = 114           # frame row stride
FROWS = 116        # 114 padded rows + 2 junk margin rows
IMGS = 2
TROWS = 4          # output rows per matmul tile
NT = TROWS * RS    # matmul free dim (456)
NTILES = H // TROWS
BAND = 28          # input rows binarized per activation op
EPS = 1e-6

F32 = mybir.dt.float32
FP8 = mybir.dt.float8e4
AF = mybir.ActivationFunctionType
DRM = mybir.MatmulPerfMode.DoubleRow
ALU = mybir.AluOpType

# tap flat offsets in raster order; pairs (0,1) (2,3) (4,5) (6,7), single 8
TAP_OFF = [dy * RS + dx for dy in (-1, 0, 1) for dx in (-1, 0, 1)]


def _split_multiwait(nc):
    """Walrus in this container rejects >1 semaphore wait per instruction
    ("Too many sync wait commands"); hoist extra waits onto preceding nops
    on the same engine."""
    import bass_rust

    for f in nc.m.functions:
        for bb in f.blocks:
            new_insts = []
            for inst in bb.instructions:
                si = inst.sync_info
                waits = list(si.on_wait) if si is not None and si.on_wait else []
                if len(waits) > 1:
                    for w in waits[:-1]:
                        nop = mybir.InstNoOp(
                            name=nc.get_next_instruction_name(),
                            engine=inst.engine,
                            ins=[],
                            outs=[],
                        )
                        nop.sync_info = bass_rust.SyncInfo(on_wait=[w], on_update=[])
                        new_insts.append(nop)
                    si.on_wait = [waits[-1]]
                    inst.sync_info = si
                new_insts.append(inst)
            bb.instructions = new_insts


def build_nc():
    nc = bass.Bass()

    x_ext = nc.declare_dram_parameter("x", [IMGS, 2 * CPB, H, W], F32, isOutput=False)
    wr_ext = nc.declare_dram_parameter("wrT", [CPB, 9 * CPB], F32, isOutput=False)
    wi_ext = nc.declare_dram_parameter("wiT", [CPB, 9 * CPB], F32, isOutput=False)
    bias_ext = nc.declare_dram_parameter("bias2", [CPB, 2], F32, isOutput=False)
    out_ext = nc.declare_dram_parameter("out", [IMGS, 2 * CPB, H, W], F32, isOutput=True)

    x_flat = x_ext.rearrange("b c h w -> (b c) h w")       # [512, 112, 112]
    out_flat = out_ext.rearrange("b c h w -> (b c) h w")

    with tile.TileContext(nc) as tc:
        with (
            tc.tile_pool(name="wstage", bufs=2) as wstage_pool,
            tc.tile_pool(name="wbin", bufs=1) as wbin_pool,
            tc.tile_pool(name="biasp", bufs=1) as bias_pool,
            tc.tile_pool(name="xq", bufs=1) as xq_pool,
            tc.tile_pool(name="stage", bufs=4) as stage_pool,
            tc.tile_pool(name="tmp", bufs=6) as tmp_pool,
            tc.tile_pool(name="outsb", bufs=8) as out_pool,
            tc.tile_pool(name="psum", bufs=8, space="PSUM") as psum_pool,
        ):
            # per-partition scalar constant for activation bias
            eps_pos = bias_pool.tile([CPB, 1], F32, tag="epsp")
            nc.gpsimd.memset(eps_pos[:], EPS)

            # HAM warmup: dummy matmuls on junk data with no dependencies so
            # the PE clock-gate reaches 8/8 before the first real matmul
            junk = bias_pool.tile([CPB, 512], FP8, tag="junk")
            nc.gpsimd.memset(junk[:, 0:1], 1.0)
            jps = psum_pool.tile([CPB, 512], F32, tag="ps", name="jps")
            for _ in range(12):
                nc.tensor.matmul(jps[:], junk[:, :CPB], junk[:], start=True,
                                 stop=True)
            for _ in range(90):
                nc.tensor.matmul(jps[:, :256], junk[:, :CPB], junk[:, :256],
                                 start=True, stop=True)
            jout = bias_pool.tile([CPB, 1], F32, tag="jout")
            nc.vector.tensor_copy(jout[:], jps[:, 0:1])

            # ---- weights ----
            wr_f32 = wstage_pool.tile([CPB, 9 * CPB], F32, tag="wstage")
            nc.sync.dma_start(wr_f32[:, :576], wr_ext[:, :576])
            nc.sync.dma_start(wr_f32[:, 576:], wr_ext[:, 576:])
            wi_f32 = wstage_pool.tile([CPB, 9 * CPB], F32, tag="wstage")
            nc.sync.dma_start(wi_f32[:, :576], wi_ext[:, :576])
            nc.sync.dma_start(wi_f32[:, 576:], wi_ext[:, 576:])

            # binarized fp8 weights [ci, tap, co]; wq_s = wq_r + wq_i
            wq_r = wbin_pool.tile([CPB, 9, CPB], FP8, tag="wqr")
            wq_i = wbin_pool.tile([CPB, 9, CPB], FP8, tag="wqi")
            wq_s = wbin_pool.tile([CPB, 9, CPB], FP8, tag="wqs")
            wr_v = wr_f32[:].rearrange("p (t c) -> p t c", c=CPB)
            wi_v = wi_f32[:].rearrange("p (t c) -> p t c", c=CPB)
            nc.scalar.activation(wq_r[:], wr_v, AF.Sign, bias=eps_pos[:], scale=1.0)

            bias_sb = bias_pool.tile([CPB, 2], F32)
            nc.sync.dma_start(bias_sb[:], bias_ext[:])
            bias_ir = bias_pool.tile([CPB, 1], F32, tag="biasir")
            nc.vector.tensor_add(bias_ir[:], bias_sb[:, 1:2], bias_sb[:, 0:1])

            # ---- persistent binarized fp8 frames ----
            # frame: [128, FROWS, RS]; frame row = padded row + 1 (1 junk
            # margin row on top); cols 0 / 113 are the zero pad columns,
            # cols 114-115 slack (only ever read into discarded pad lanes)
            def frame(nm):
                return xq_pool.tile([CPB, FROWS, RS], FP8, tag=nm, name=nm)

            xqr = [frame(f"xqr{i}") for i in range(IMGS)]
            xqi = [frame(f"xqi{i}") for i in range(IMGS)]
            xqs = [frame(f"xqs{i}") for i in range(IMGS)]
            for i in range(IMGS):
                eng = nc.vector if i == 0 else nc.gpsimd
                for t in (xqr[i], xqi[i], xqs[i]):
                    eng.memset(t[:, 1:2, :], 0.0)          # padded row 0
                    eng.memset(t[:, 114:115, :], 0.0)      # padded row 113
                    eng.memset(t[:, 1:115, 0:1], 0.0)      # padded col 0
                    eng.memset(t[:, 1:115, 113:114], 0.0)  # padded col 113

            flat = {}
            for i in range(IMGS):
                flat[("r", i)] = xqr[i][:].rearrange("p r c -> p (r c)")
                flat[("i", i)] = xqi[i][:].rearrange("p r c -> p (r c)")
                flat[("s", i)] = xqs[i][:].rearrange("p r c -> p (r c)")

            # ---- binarize input + build the sum frame, band by band ----
            def binarize_rows(img, r0, nr):
                rows = slice(r0 + 2, r0 + 2 + nr)
                for cib, dst in ((0, xqr), (1, xqi)):
                    ch0 = img * 2 * CPB + cib * CPB
                    st = stage_pool.tile([CPB, BAND, W], F32, tag="stage")
                    nc.sync.dma_start(
                        st[:, :nr, :],
                        x_flat[ch0:ch0 + CPB, r0:r0 + nr, :],
                    )
                    nc.scalar.activation(
                        dst[img][:, rows, 1:113], st[:, :nr, :],
                        AF.Sign, bias=eps_pos[:], scale=1.0,
                    )
                nc.vector.tensor_tensor(
                    xqs[img][:, rows, 1:113],
                    xqr[img][:, rows, 1:113],
                    xqi[img][:, rows, 1:113],
                    op=ALU.add,
                )

            def binarize_band(img, b, strips=1):
                r0 = b * BAND
                step = BAND // strips
                for s in range(strips):
                    binarize_rows(img, r0 + s * step, step)

            def conv_tiles(img, tiles):
                for t in tiles:
                    base = (4 * t + 2) * RS
                    pk = {}
                    def conv(kind):
                        w3 = {"r": wq_r, "i": wq_i, "s": wq_s}[kind]
                        xf = flat[(kind, img)]
                        ps = psum_pool.tile([CPB, NT], F32, tag="ps",
                                            name=f"ps_{kind}{img}_{t}")
                        pk[kind] = ps
                        part = [list(xf.ap)[0][0], CPB]
                        for p in range(4):
                            o0, o1 = TAP_OFF[2 * p], TAP_OFF[2 * p + 1]
                            rhs = bass.AP(
                                xf.tensor, xf.offset + o0 + base,
                                [part, [o1 - o0, 2], [1, NT]],
                            )
                            nc.tensor.matmul(
                                ps[:], w3[:, 2 * p:2 * p + 2, :], rhs,
                                start=(p == 0), stop=False, perf_mode=DRM,
                            )
                        nc.tensor.matmul(
                            ps[:], w3[:, 8, :],
                            xf[:, base + TAP_OFF[8]:base + TAP_OFF[8] + NT],
                            start=False, stop=True,
                        )
                        return ps

                    # out_real = A - B + bias_r ; out_imag = C - A - B + bias_i
                    # ScalarE (fast PSUM port) evacuates each bank compactly
                    # right after its conv, exactly one reader per bank:
                    #   An2 = A + bias_r ; Bn0 = -B
                    # then out_real = An2 + Bn0 (SBUF-only, GpSimd)
                    #      out_imag = ((C - An2) + (bias_i+bias_r)) + Bn0
                    A = conv("r")
                    An2 = tmp_pool.tile([CPB, TROWS, W], F32, tag="An")
                    Av = A[:].rearrange("p (r c) -> p r c", c=RS)
                    nc.scalar.activation(An2[:], Av[:, :, 1:113], AF.Identity,
                                         bias=bias_sb[:, 0:1], scale=1.0)
                    Bp = conv("i")
                    Bn0 = tmp_pool.tile([CPB, TROWS, W], F32, tag="Bn")
                    Bv = Bp[:].rearrange("p (r c) -> p r c", c=RS)
                    nc.scalar.activation(Bn0[:], Bv[:, :, 1:113], AF.Identity,
                                         bias=0.0, scale=-1.0)
                    C = conv("s")
                    Cv = C[:].rearrange("p (r c) -> p r c", c=RS)

                    osb = out_pool.tile([CPB, 2, TROWS, W], F32, tag="osb")
                    nc.gpsimd.tensor_tensor(osb[:, 0], An2[:], Bn0[:], op=ALU.add)
                    t5 = tmp_pool.tile([CPB, TROWS, W], F32, tag="t5")
                    nc.vector.tensor_sub(t5[:], Cv[:, :, 1:113], An2[:])
                    nc.vector.scalar_tensor_tensor(
                        osb[:, 1], t5[:], bias_ir[:], Bn0[:],
                        op0=ALU.add, op1=ALU.add,
                    )

                    # one DMA for both channel halves: dst walks [ch-within-
                    # block, block, row, col] to match the tile's layout
                    dst = bass.AP(
                        out_flat.tensor,
                        img * 2 * CPB * H * W + 4 * t * W,
                        [[H * W, CPB], [CPB * H * W, 2], [W, TROWS], [1, W]],
                    )
                    nc.sync.dma_start(dst, osb[:])

            # tile t needs input rows <= 4t+4; band b supplies rows < 28(b+1).
            # Binarize lands in 14-row strips, interleaved BETWEEN tiles so
            # the long Sign ops never head-of-line-block the short PSUM
            # evacuations in the static ScalarE queue; strips stay three
            # tile-groups ahead of their consumers.
            ranges = [range(0, 6), range(6, 13), range(13, 20), range(20, 28)]
            groups = [(i, b) for i in range(IMGS) for b in range(H // BAND)]
            binarize_rows(0, 0, 14)
            nc.scalar.activation(wq_i[:], wi_v, AF.Sign, bias=eps_pos[:], scale=1.0)
            nc.vector.tensor_tensor(wq_s[:], wq_r[:], wq_i[:], op=ALU.add)
            binarize_rows(0, 14, 14)
            for b in (1, 2):
                binarize_band(0, b, strips=2)
            for gi, (img, b) in enumerate(groups):
                tiles = list(ranges[b])
                ahead = groups[gi + 3] if gi + 3 < len(groups) else None
                conv_tiles(img, tiles[:2])
                if ahead:
                    binarize_rows(ahead[0], ahead[1] * BAND, 14)
                conv_tiles(img, tiles[2:4])
                if ahead:
                    binarize_rows(ahead[0], ahead[1] * BAND + 14, 14)
                conv_tiles(img, tiles[4:])

    _split_multiwait(nc)
    return nc


def _prep(x, weight_real, weight_imag, bias):
    x = np.ascontiguousarray(np.asarray(x, dtype=np.float32))
    wr = np.asarray(weight_real, dtype=np.float32)
    wi = np.asarray(weight_imag, dtype=np.float32)
    bias = np.asarray(bias, dtype=np.float32)
    wrT = np.ascontiguousarray(wr.transpose(1, 2, 3, 0).reshape(CPB, 9 * CPB))
    wiT = np.ascontiguousarray(wi.transpose(1, 2, 3, 0).reshape(CPB, 9 * CPB))
    bias2 = np.ascontiguousarray(bias.reshape(2, CPB).T)
    return [
        {"x": x[IMGS * c:IMGS * (c + 1)], "wrT": wrT, "wiT": wiT, "bias2": bias2}
        for c in range(N_CORES)
    ]


def kernel(x, weight_real, weight_imag, bias):
    in_maps = _prep(x, weight_real, weight_imag, bias)
    nc = build_nc()
    res = run_bass_kernel_spmd(nc, in_maps, core_ids=list(range(N_CORES)))
    return np.concatenate([res.results[i]["out"] for i in range(N_CORES)], axis=0)


def run_traced(x, weight_real, weight_imag, bias, **trace_kwargs):
    """test.py entry: same as kernel() but with neuron-profile tracing."""
    in_maps = _prep(x, weight_real, weight_imag, bias)
    nc = build_nc()
    res = run_bass_kernel_spmd(
        nc, in_maps, core_ids=list(range(N_CORES)), trace=True, **trace_kwargs
    )
    out = np.concatenate([res.results[i]["out"] for i in range(N_CORES)], axis=0)
    return out, res
